# revision 6
# baseline (speedup 1.0000x reference)
"""Trainium2 Bass kernel for nn_Aggregation (SAN-style position-dependent
3x3 depthwise aggregation with share_planes=8).

  out[n, c, h, w] = sum_k input[n, c, h+dh(k), w+dw(k)] * weight[n, c//8, k, h*W+w]

Sharding: data-parallel over batch N=8 across the 8 NeuronCores (one image
per core, no collectives).

Per-core design (input [256,56,56], weight [32,9,3136] per image):
  - SBUF partition p = q*32 + g: q in 0..3 = 14-row quarter of the image,
    g in 0..31 = weight group. The 8 share-channels of a group live in the
    free dimension, so each weight element is read via a stride-0 broadcast
    AP instead of being replicated.
  - The host pre-packs, per partition, a zero-padded flat image slab
    (1 guard + 16 rows [14 + 2 halo] * 56 cols + 1 guard + 1 pad = 900 per
    share-channel), so each tap (dh, dw) is a single contiguous 784-slice
    at offset 1 + (dh+1)*56 + dw. Column wrap-around reads are neutralized
    by zeroing the weight's edge columns host-side (those taps multiply
    out-of-image zero padding in the exact computation). Host packing also
    makes every DMA one big contiguous span per partition (descriptor-
    efficient) and removes all on-chip memsets.
  - fp16 storage: DVE tensor_tensor runs in 2x perf mode (needs 16-bit,
    step 1, 4B-aligned APs -> a second, one-element-shifted read of the x
    slab gives every tap an even base offset). ALU math is fp32 internally.
  - Compute: 9 taps: tensor_mul into acc (first) / tmp + tensor_add (rest),
    optionally split along the share axis to overlap DMA with compute.
"""

import numpy as np

N, C, H, W = 8, 256, 56, 56
G, KK, L = 32, 9, 3136          # weight groups, taps, spatial
SHARE = 8                        # C // G
Q = 4                            # row-quarters
RQ = H // Q                      # 14 rows per quarter
LQ = RQ * W                      # 784 pixels per quarter
XA = 900                         # guard + 16*56 + guard + pad (even)

DTYPE = "float16"                # on-chip storage dtype
SPLIT = 4                        # share-axis chunks (overlap DMA/compute)

_CACHE = {}


def _build():
    import concourse.bacc as bacc
    import concourse.mybir as mybir
    import concourse.tile as tile

    dt = getattr(mybir.dt, DTYPE)

    nc = bacc.Bacc("TRN2", target_bir_lowering=False, debug=False)
    xin = nc.dram_tensor("xin", [128, SHARE, XA], dt, kind="ExternalInput")
    win = nc.dram_tensor("win", [128, KK, LQ], dt, kind="ExternalInput")
    out = nc.dram_tensor("out", [128, SHARE, LQ], dt, kind="ExternalOutput")

    schunks = []
    step = SHARE // SPLIT
    for i in range(SPLIT):
        schunks.append((i * step, (i + 1) * step))

    with tile.TileContext(nc) as tc:
        with tc.tile_pool(name="main", bufs=1) as pool:
            xa = pool.tile([128, SHARE, XA], dt)
            xb = pool.tile([128, SHARE, XA - 4], dt)
            wt = pool.tile([128, KK, LQ], dt)
            acc = pool.tile([128, SHARE, LQ], dt)
            tmp = pool.tile([128, SHARE, LQ], dt)

            # weight on the scalar HWDGE engine (3 k-plane groups so early taps
            # unblock sooner), x slabs on the sync HWDGE engine — the ~1us
            # per-dma_start queue-arming costs then overlap across engines.
            nc.scalar.dma_start(out=wt[:], in_=win.ap())
            for s0, s1 in schunks:
                nc.sync.dma_start(out=xa[:, s0:s1, :], in_=xin.ap()[:, s0:s1, :])
                nc.sync.dma_start(
                    out=xb[:, s0:s1, :], in_=xin.ap()[:, s0:s1, 1 : XA - 3]
                )

            for s0, s1 in schunks:
                ns = s1 - s0
                for k in range(KK):
                    dh, dw = k // 3 - 1, k % 3 - 1
                    if dw == 0:
                        base = (dh + 1) * W      # even; xb = xa shifted by 1
                        x_ap = xb[:, s0:s1, base : base + LQ]
                    else:
                        base = 1 + (dh + 1) * W + dw  # even by construction
                        x_ap = xa[:, s0:s1, base : base + LQ]
                    w_ap = wt[:, k : k + 1, :].broadcast_to([128, ns, LQ])
                    if k == 0:
                        nc.vector.tensor_mul(acc[:, s0:s1, :], x_ap, w_ap)
                    else:
                        nc.vector.tensor_mul(tmp[:, s0:s1, :], x_ap, w_ap)
                        nc.vector.tensor_add(
                            acc[:, s0:s1, :], acc[:, s0:s1, :], tmp[:, s0:s1, :]
                        )
                nc.scalar.dma_start(out=out.ap()[:, s0:s1, :], in_=acc[:, s0:s1, :])

    nc.compile()
    return nc


def _get_nc():
    if "nc" not in _CACHE:
        _CACHE["nc"] = _build()
    return _CACHE["nc"]


def _prep_shards(input, weight):
    np_dt = np.dtype(DTYPE)
    # padded image per (g, s): rows -1..56 zero-padded
    inp = np.asarray(input).reshape(N, G, SHARE, H, W)
    pad = np.zeros((N, G, SHARE, H + 2, W), dtype=np_dt)
    pad[:, :, :, 1 : H + 1, :] = inp
    # x slab: [N, q, g, s, XA]
    xh = np.zeros((N, Q, G, SHARE, XA), dtype=np_dt)
    for q in range(Q):
        xh[:, q, :, :, 1 : 1 + 16 * W] = pad[:, :, :, q * RQ : q * RQ + 16, :].reshape(
            N, G, SHARE, 16 * W
        )
    xh = xh.reshape(N, 128, SHARE, XA)

    # weight: [N, q, g, k, LQ] with out-of-image edge columns zeroed
    wh = np.asarray(weight).astype(np_dt).reshape(N, G, KK, H, W)
    for k in range(KK):
        dwk = k % 3 - 1
        if dwk == -1:
            wh[:, :, k, :, 0] = 0
        elif dwk == 1:
            wh[:, :, k, :, W - 1] = 0
    wh = (
        wh.reshape(N, G, KK, Q, LQ)
        .transpose(0, 3, 1, 2, 4)
        .reshape(N, 128, KK, LQ)
    )
    return [
        {"xin": np.ascontiguousarray(xh[n]), "win": np.ascontiguousarray(wh[n])}
        for n in range(N)
    ]


def _unpack_out(res_list):
    # res: [128, SHARE, LQ] per core -> (N, C, H, W) float32
    o = np.stack([r["out"] for r in res_list], axis=0).astype(np.float32)
    o = o.reshape(N, Q, G, SHARE, LQ).transpose(0, 2, 3, 1, 4)
    return np.ascontiguousarray(o.reshape(N, C, H, W))


def kernel(input, weight):
    from concourse.bass_utils import run_bass_kernel_spmd

    nc = _get_nc()
    in_maps = _prep_shards(input, weight)
    res = run_bass_kernel_spmd(nc, in_maps, core_ids=list(range(N)))
    return _unpack_out(res.results)


# revision 8
# speedup vs baseline: 1.0461x; 1.0461x over previous
"""Trainium2 Bass kernel for nn_Aggregation (SAN-style position-dependent
3x3 depthwise aggregation with share_planes=8).

  out[n, c, h, w] = sum_k input[n, c, h+dh(k), w+dw(k)] * weight[n, c//8, k, h*W+w]

Sharding: data-parallel over batch N=8 across the 8 NeuronCores (one image
per core, no collectives).

Per-core design (input [256,56,56], weight [32,9,3136] per image):
  - SBUF partition p = q*32 + g: q in 0..3 = 14-row quarter of the image,
    g in 0..31 = weight group. The 8 share-channels of a group live in the
    free dimension, so each weight element is read via a stride-0 broadcast
    AP instead of being replicated.
  - The host pre-packs, per partition, a zero-padded flat image slab
    (1 guard + 16 rows [14 + 2 halo] * 56 cols + 1 guard + 1 pad = 900 per
    share-channel), so each tap (dh, dw) is a single contiguous 784-slice
    at offset 1 + (dh+1)*56 + dw. Column wrap-around reads are neutralized
    by zeroing the weight's edge columns host-side (those taps multiply
    out-of-image zero padding in the exact computation). Host packing also
    makes every DMA one big contiguous span per partition (descriptor-
    efficient) and removes all on-chip memsets.
  - fp16 storage: DVE tensor_tensor runs in 2x perf mode (needs 16-bit,
    step 1, 4B-aligned APs -> a second, one-element-shifted read of the x
    slab gives every tap an even base offset). ALU math is fp32 internally.
  - Compute: 9 taps: tensor_mul into acc (first) / tmp + tensor_add (rest),
    optionally split along the share axis to overlap DMA with compute.
"""

import numpy as np

N, C, H, W = 8, 256, 56, 56
G, KK, L = 32, 9, 3136          # weight groups, taps, spatial
SHARE = 8                        # C // G
Q = 4                            # row-quarters
RQ = H // Q                      # 14 rows per quarter
LQ = RQ * W                      # 784 pixels per quarter
XA = 900                         # guard + 16*56 + guard + pad (even)

DTYPE = "float16"                # on-chip storage dtype
SPLIT = 2                        # share-axis chunks (overlap DMA/compute)
# tap order: xa-based taps (dw=+-1) first so compute can start before the
# on-chip xb shift-copies finish; xb-based taps (dw=0) last.
TAP_ORDER = [0, 2, 3, 5, 6, 8, 1, 4, 7]

_CACHE = {}


def _build():
    import concourse.bacc as bacc
    import concourse.mybir as mybir
    import concourse.tile as tile

    dt = getattr(mybir.dt, DTYPE)

    nc = bacc.Bacc("TRN2", target_bir_lowering=False, debug=False)
    xin = nc.dram_tensor("xin", [128, SHARE, XA], dt, kind="ExternalInput")
    win = nc.dram_tensor("win", [128, KK, LQ], dt, kind="ExternalInput")
    out = nc.dram_tensor("out", [128, SHARE, LQ], dt, kind="ExternalOutput")

    schunks = []
    step = SHARE // SPLIT
    for i in range(SPLIT):
        schunks.append((i * step, (i + 1) * step))

    with tile.TileContext(nc) as tc:
        with tc.tile_pool(name="main", bufs=1) as pool:
            xa = pool.tile([128, SHARE, XA], dt)
            xb = pool.tile([128, SHARE, XA - 4], dt)
            wt = pool.tile([128, KK, LQ], dt)
            acc = pool.tile([128, SHARE, LQ], dt)
            tmp = pool.tile([128, SHARE, LQ], dt)

            # weight (3 k-plane groups so early taps unblock sooner) on the
            # scalar HWDGE engine, x slabs on the sync HWDGE engine — the ~1us
            # per-dma_start queue-arming costs then overlap across engines.
            # xb (the one-element-shifted slab that gives dw=0 taps an even
            # base) is built on-chip by the otherwise-idle ScalarE instead of
            # being DMA'd a second time from HBM.
            for k0 in range(0, KK, 3):
                nc.scalar.dma_start(
                    out=wt[:, k0 : k0 + 3, :], in_=win.ap()[:, k0 : k0 + 3, :]
                )
            for s0, s1 in schunks:
                nc.sync.dma_start(out=xa[:, s0:s1, :], in_=xin.ap()[:, s0:s1, :])
            for s0, s1 in schunks:
                nc.scalar.copy(xb[:, s0:s1, :], xa[:, s0:s1, 1 : XA - 3])

            for s0, s1 in schunks:
                ns = s1 - s0
                first = True
                for k in TAP_ORDER:
                    dh, dw = k // 3 - 1, k % 3 - 1
                    if dw == 0:
                        base = (dh + 1) * W      # even; xb = xa shifted by 1
                        x_ap = xb[:, s0:s1, base : base + LQ]
                    else:
                        base = 1 + (dh + 1) * W + dw  # even by construction
                        x_ap = xa[:, s0:s1, base : base + LQ]
                    w_ap = wt[:, k : k + 1, :].broadcast_to([128, ns, LQ])
                    if first:
                        nc.vector.tensor_mul(acc[:, s0:s1, :], x_ap, w_ap)
                        first = False
                    else:
                        nc.vector.tensor_mul(tmp[:, s0:s1, :], x_ap, w_ap)
                        nc.vector.tensor_add(
                            acc[:, s0:s1, :], acc[:, s0:s1, :], tmp[:, s0:s1, :]
                        )
                nc.sync.dma_start(out=out.ap()[:, s0:s1, :], in_=acc[:, s0:s1, :])

    nc.compile()
    return nc


def _get_nc():
    if "nc" not in _CACHE:
        _CACHE["nc"] = _build()
    return _CACHE["nc"]


def _prep_shards(input, weight):
    np_dt = np.dtype(DTYPE)
    # padded image per (g, s): rows -1..56 zero-padded
    inp = np.asarray(input).reshape(N, G, SHARE, H, W)
    pad = np.zeros((N, G, SHARE, H + 2, W), dtype=np_dt)
    pad[:, :, :, 1 : H + 1, :] = inp
    # x slab: [N, q, g, s, XA]
    xh = np.zeros((N, Q, G, SHARE, XA), dtype=np_dt)
    for q in range(Q):
        xh[:, q, :, :, 1 : 1 + 16 * W] = pad[:, :, :, q * RQ : q * RQ + 16, :].reshape(
            N, G, SHARE, 16 * W
        )
    xh = xh.reshape(N, 128, SHARE, XA)

    # weight: [N, q, g, k, LQ] with out-of-image edge columns zeroed
    wh = np.asarray(weight).astype(np_dt).reshape(N, G, KK, H, W)
    for k in range(KK):
        dwk = k % 3 - 1
        if dwk == -1:
            wh[:, :, k, :, 0] = 0
        elif dwk == 1:
            wh[:, :, k, :, W - 1] = 0
    wh = (
        wh.reshape(N, G, KK, Q, LQ)
        .transpose(0, 3, 1, 2, 4)
        .reshape(N, 128, KK, LQ)
    )
    return [
        {"xin": np.ascontiguousarray(xh[n]), "win": np.ascontiguousarray(wh[n])}
        for n in range(N)
    ]


def _unpack_out(res_list):
    # res: [128, SHARE, LQ] per core -> (N, C, H, W) float32
    o = np.stack([r["out"] for r in res_list], axis=0).astype(np.float32)
    o = o.reshape(N, Q, G, SHARE, LQ).transpose(0, 2, 3, 1, 4)
    return np.ascontiguousarray(o.reshape(N, C, H, W))


def kernel(input, weight):
    from concourse.bass_utils import run_bass_kernel_spmd

    nc = _get_nc()
    in_maps = _prep_shards(input, weight)
    res = run_bass_kernel_spmd(nc, in_maps, core_ids=list(range(N)))
    return _unpack_out(res.results)
